# revision 4
# baseline (speedup 1.0000x reference)
"""Causal multi-head attention block (QKV proj -> attention -> out proj) on 8
Trainium2 NeuronCores.

Sharding: core i handles batch b = i//2 and head-group g = i%2 (6 of 12 heads).
Each core computes its heads' attention output and a partial output projection
(rows g*384:(g+1)*384 of w_proj); the host sums the two partials per batch and
adds b_proj.

On-core dataflow (per core):
  x^T tiles  [c,t]   via DMA-transpose (bf16)
  q^T, k^T   [n,t]   = w_q/k^T-stationary matmuls, pair-stacked 2 heads/tile
  v          [t,n]   natural layout
  S^T        [kt,qt] = k^T-stationary matmul, row-packed pairs (K=64 halves)
  P^T        = exp(S^T)  (no max subtraction: |scores| is O(10), safe in fp32)
  out^T      [d,qt]  = v-stationary matmul, col-packed pairs into one PSUM bank
  l          [1,qt]  = ones-stationary matmul rows 0/32 of one PSUM bank
  out^T * (1/l)      -> ao^T [f,t], fed as lhsT to the projection matmul
Causal masking: only kt-tiles <= qt are computed; diagonal 128x128 blocks get a
triangular -1e9 mask added pre-exp; fully-masked column ranges are skipped via
partial-free-dim matmuls accumulating into the same PSUM region.
"""

import math
from contextlib import ExitStack

import numpy as np
import ml_dtypes

import concourse.bass as bass
import concourse.mybir as mybir
import concourse.tile as tile
from concourse import bacc, library_config
from concourse.bass_utils import run_bass_kernel_spmd

B, T_FULL, C = 4, 2048, 768
NH, HD = 12, 64
HL = NH // 2            # heads per core
NPAIR = HL // 2         # head pairs per core
NQK = HL * HD           # 384 features per core for each of q/k/v
N_CORES = 8
P = 128
SW = 512                # qt strip width
NC_T = C // P           # 6 contraction tiles
F32 = mybir.dt.float32
BF16 = mybir.dt.bfloat16
NPF = np.float32
NPBF = ml_dtypes.bfloat16

_CACHE: dict = {}


def build(T: int = T_FULL):
    NT = T // P
    NSTRIP = T // SW
    nc = bacc.Bacc("TRN2", target_bir_lowering=False, debug=False,
                   num_devices=N_CORES)
    x_d = nc.dram_tensor("x", [T, C], BF16, kind="ExternalInput")
    w_d = nc.dram_tensor("wqkv", [C, 3 * NQK], BF16, kind="ExternalInput")
    bqk_d = nc.dram_tensor("bqk", [P, 2 * NPAIR], F32, kind="ExternalInput")
    bv_d = nc.dram_tensor("bv", [1, NQK], F32, kind="ExternalInput")
    wp_d = nc.dram_tensor("wp", [NQK, C], BF16, kind="ExternalInput")
    tri_d = nc.dram_tensor("tri", [P, P], F32, kind="ExternalInput")
    out_d = nc.dram_tensor("out", [T, C], F32, kind="ExternalOutput")

    EXP = mybir.ActivationFunctionType.Exp
    ADD = mybir.AluOpType.add
    MUL = mybir.AluOpType.mult

    with ExitStack() as ctx:
        tc = ctx.enter_context(tile.TileContext(nc))
        persist = ctx.enter_context(tc.tile_pool(name="persist", bufs=1))
        ppool = ctx.enter_context(tc.tile_pool(name="pt", bufs=6))
        smallp = ctx.enter_context(tc.tile_pool(name="small", bufs=4))
        outp = ctx.enter_context(tc.tile_pool(name="outsb", bufs=3))
        ps_s = ctx.enter_context(tc.tile_pool(name="ps_s", bufs=2, space="PSUM"))
        ps_q = ctx.enter_context(tc.tile_pool(name="ps_q", bufs=2, space="PSUM"))
        ps_av = ctx.enter_context(tc.tile_pool(name="ps_av", bufs=1, space="PSUM"))
        ps_l = ctx.enter_context(tc.tile_pool(name="ps_l", bufs=1, space="PSUM"))

        nc.gpsimd.load_library(library_config.attn)

        # ---- persistent inputs ----
        xT = persist.tile([P, NC_T, T], BF16)
        for cb in range(NC_T):
            nc.sync.dma_start_transpose(xT[:, cb, :],
                                        x_d.ap()[:, cb * P:(cb + 1) * P])
        w_sb = persist.tile([P, NC_T, 3 * NQK], BF16)
        nc.sync.dma_start(w_sb[:], w_d.ap().rearrange("(a p) n -> p a n", p=P))
        wp_sb = persist.tile([P, NQK // P, C], BF16)
        nc.sync.dma_start(wp_sb[:], wp_d.ap().rearrange("(a p) n -> p a n", p=P))
        bqk_sb = persist.tile([P, 2 * NPAIR], F32)
        nc.sync.dma_start(bqk_sb[:], bqk_d.ap())
        bv_bc = persist.tile([P, NQK], F32)
        bv_ap = bv_d.ap()
        bv_bcast = bass.AP(tensor=bv_ap.tensor, offset=bv_ap.offset,
                           ap=[[0, P], [1, NQK]])
        nc.sync.dma_start(bv_bc[:], bv_bcast)
        tri_sb = persist.tile([P, P], F32)
        nc.sync.dma_start(tri_sb[:], tri_d.ap())
        ones_sb = persist.tile([P, 1], BF16)
        nc.vector.memset(ones_sb[:], 1.0)

        qT = persist.tile([P, NPAIR, T], BF16)
        kT = persist.tile([P, NPAIR, T], BF16)
        v_sb = persist.tile([P, NT, NQK], BF16)
        aoT = persist.tile([P, NPAIR, T], BF16)

        for p in range(NPAIR):
            # ---- QKV for this pair ----
            for which, dst in ((p, qT), (NPAIR + p, kT)):
                for s in range(NSTRIP):
                    ps_t = ps_q.tile([P, SW], F32, tag="q")
                    for cb in range(NC_T):
                        nc.tensor.matmul(
                            ps_t[:], w_sb[:, cb, which * P:(which + 1) * P],
                            xT[:, cb, s * SW:(s + 1) * SW],
                            start=(cb == 0), stop=(cb == NC_T - 1))
                    nc.vector.tensor_scalar_add(
                        dst[:, p, s * SW:(s + 1) * SW], ps_t[:],
                        bqk_sb[:, which:which + 1])
            for tt in range(NT):
                ps_t = ps_q.tile([P, SW], F32, tag="q")
                vcols = 2 * NQK + p * P
                for cb in range(NC_T):
                    nc.tensor.matmul(
                        ps_t[:, 0:P], xT[:, cb, tt * P:(tt + 1) * P],
                        w_sb[:, cb, vcols:vcols + P],
                        start=(cb == 0), stop=(cb == NC_T - 1))
                nc.vector.tensor_tensor(
                    out=v_sb[:, tt, p * P:(p + 1) * P], in0=ps_t[:, 0:P],
                    in1=bv_bc[:, p * P:(p + 1) * P], op=ADD)

            # ---- attention for this pair ----
            hA, hB = 2 * p, 2 * p + 1
            for s in range(NSTRIP):
                n_kt = 4 * (s + 1)
                av_ps = ps_av.tile([P, SW], F32, tag="av")
                l_ps = ps_l.tile([P, SW], F32, tag="l")
                for g in range(n_kt // 2):
                    sA = ps_s.tile([P, 2, SW], F32, tag="s")
                    sB = ps_s.tile([P, 2, SW], F32, tag="s")
                    for sub in range(2):
                        kt = 2 * g + sub
                        c0 = max(kt - 4 * s, 0) * P
                        kts = slice(kt * P, (kt + 1) * P)
                        qts = slice(s * SW + c0, (s + 1) * SW)
                        nc.tensor.matmul(sA[:, sub, c0:SW], kT[0:64, p, kts],
                                         qT[0:64, p, qts], start=True, stop=True)
                        nc.tensor.matmul(sB[:, sub, c0:SW], kT[64:P, p, kts],
                                         qT[64:P, p, qts], start=True, stop=True)
                        if kt - 4 * s >= 0:
                            for st in (sA, sB):
                                nc.vector.tensor_tensor(
                                    out=st[:, sub, c0:c0 + P],
                                    in0=st[:, sub, c0:c0 + P],
                                    in1=tri_sb[:], op=ADD)
                    pA = ppool.tile([P, 2, SW], BF16, tag="pt")
                    pB = ppool.tile([P, 2, SW], BF16, tag="pt")
                    nc.scalar.activation(pA[:], sA[:], EXP)
                    nc.scalar.activation(pB[:], sB[:], EXP)
                    for sub in range(2):
                        kt = 2 * g + sub
                        c0 = max(kt - 4 * s, 0) * P
                        first, last = kt == 0, kt == n_kt - 1
                        nc.tensor.matmul(
                            av_ps[0:64, c0:SW], v_sb[:, kt, hA * HD:(hA + 1) * HD],
                            pA[:, sub, c0:SW], start=first, stop=last,
                            tile_position=(0, 0), skip_group_check=True)
                        nc.tensor.matmul(
                            av_ps[64:P, c0:SW], v_sb[:, kt, hB * HD:(hB + 1) * HD],
                            pB[:, sub, c0:SW], start=first, stop=last,
                            tile_position=(0, 64), skip_group_check=True)
                        nc.tensor.matmul(
                            l_ps[0:1, c0:SW], ones_sb[:], pA[:, sub, c0:SW],
                            start=first, stop=last, tile_position=(0, 0),
                            skip_group_check=True)
                        nc.tensor.matmul(
                            l_ps[32:33, c0:SW], ones_sb[:], pB[:, sub, c0:SW],
                            start=first, stop=last, tile_position=(0, 32),
                            skip_group_check=True)
                rlA = smallp.tile([1, SW], F32, tag="rl")
                rlB = smallp.tile([1, SW], F32, tag="rl")
                nc.vector.reciprocal(rlA[:], l_ps[0:1, :])
                nc.vector.reciprocal(rlB[:], l_ps[32:33, :])
                # partition_broadcast only writes starting at partition 0
                rbA = smallp.tile([64, SW], F32, tag="rb")
                rbB = smallp.tile([64, SW], F32, tag="rb")
                nc.gpsimd.partition_broadcast(rbA[:], rlA[:], channels=64)
                nc.gpsimd.partition_broadcast(rbB[:], rlB[:], channels=64)
                ss = slice(s * SW, (s + 1) * SW)
                nc.vector.tensor_tensor(out=aoT[0:64, p, ss],
                                        in0=av_ps[0:64, :], in1=rbA[:], op=MUL)
                nc.vector.tensor_tensor(out=aoT[64:P, p, ss],
                                        in0=av_ps[64:P, :], in1=rbB[:], op=MUL)

        # ---- output projection (partial: this core's feature rows) ----
        for tt in range(NT):
            pr = ps_s.tile([P, 2, SW], F32, tag="s")
            tts = slice(tt * P, (tt + 1) * P)
            for ft in range(NQK // P):
                nc.tensor.matmul(pr[:, 0, :], aoT[:, ft, tts],
                                 wp_sb[:, ft, 0:SW],
                                 start=(ft == 0), stop=(ft == NQK // P - 1))
                nc.tensor.matmul(pr[:, 1, 0:C - SW], aoT[:, ft, tts],
                                 wp_sb[:, ft, SW:C],
                                 start=(ft == 0), stop=(ft == NQK // P - 1))
            ot = outp.tile([P, C], F32)
            nc.vector.tensor_copy(ot[:, 0:SW], pr[:, 0, :])
            nc.vector.tensor_copy(ot[:, SW:C], pr[:, 1, 0:C - SW])
            nc.sync.dma_start(out_d.ap()[tts, :], ot[:])

    nc.compile()
    return nc


def make_in_maps(x, w_attn, b_attn, w_proj):
    """Shard the full inputs into per-core input maps (host side)."""
    scale = 1.0 / math.sqrt(HD)
    tri = np.where(np.arange(P)[:, None] <= np.arange(P)[None, :],
                   0.0, -1e9).astype(NPF)
    in_maps = []
    for core in range(N_CORES):
        b, g = divmod(core, 2)
        cs = slice(g * NQK, (g + 1) * NQK)
        wq = w_attn[:, 0 * C:1 * C][:, cs] * scale
        wk = w_attn[:, 1 * C:2 * C][:, cs]
        wv = w_attn[:, 2 * C:3 * C][:, cs]
        wqkv = np.concatenate([wq, wk, wv], axis=1).astype(NPBF)
        bq = b_attn[0 * C:1 * C][cs] * scale
        bk = b_attn[1 * C:2 * C][cs]
        bqk = np.ascontiguousarray(
            np.concatenate([bq, bk]).reshape(2 * NPAIR, P).T).astype(NPF)
        bv = b_attn[2 * C:3 * C][cs].astype(NPF).reshape(1, NQK)
        wp = w_proj[g * NQK:(g + 1) * NQK, :].astype(NPBF)
        in_maps.append({
            "x": np.ascontiguousarray(x[b]).astype(NPBF),
            "wqkv": wqkv, "bqk": bqk, "bv": bv, "wp": wp, "tri": tri,
        })
    return in_maps


def combine_outputs(results, b_proj):
    outs = [results[i]["out"] for i in range(N_CORES)]
    out = np.stack([outs[2 * b] + outs[2 * b + 1] for b in range(B)])
    return (out + b_proj[None, None, :].astype(NPF)).astype(NPF)


def kernel(x, w_attn, b_attn, w_proj, b_proj):
    x = np.asarray(x, dtype=NPF)
    w_attn = np.asarray(w_attn, dtype=NPF)
    b_attn = np.asarray(b_attn, dtype=NPF)
    w_proj = np.asarray(w_proj, dtype=NPF)
    b_proj = np.asarray(b_proj, dtype=NPF)
    if "nc" not in _CACHE:
        _CACHE["nc"] = build(T_FULL)
    nc = _CACHE["nc"]
    in_maps = make_in_maps(x, w_attn, b_attn, w_proj)
    res = run_bass_kernel_spmd(nc, in_maps, list(range(N_CORES)))
    return combine_outputs(res.results, b_proj)


# revision 10
# speedup vs baseline: 1.0786x; 1.0786x over previous
"""Causal multi-head attention block (QKV proj -> attention -> out proj) on 8
Trainium2 NeuronCores.

Sharding: core i handles batch b = i//2 and head-group g = i%2 (6 of 12 heads).
Each core computes its heads' attention output and a partial output projection
(rows g*384:(g+1)*384 of w_proj); the host sums the two partials per batch and
adds b_proj.

On-core dataflow (per core):
  x^T tiles  [c,t]   via DMA-transpose (bf16)
  q^T, k^T   [n,t]   = w-stationary matmuls, pair-stacked 2 heads/tile
  v          [t,n]   natural layout, with a ones column appended per head
  S^T        [kt,qt] = k^T-stationary matmul, row-packed pairs (K=64 halves)
  P^T        = exp(S^T)  (no max subtraction: |scores| is O(10), safe in fp32)
  [out^T; l] [65,qt] = [v|1]-stationary matmul per head (l = softmax denom)
  out^T * (1/l)      -> ao^T [f,t], fed as lhsT to the projection matmul
Causal masking: only kt-tiles <= qt are computed; diagonal 128x128 blocks get a
0/1 triangular mask multiplied into P^T post-exp (on gpsimd); fully-masked
column ranges are skipped via partial-free-dim matmuls into the same PSUM bank.
"""

import math
from contextlib import ExitStack

import numpy as np
import ml_dtypes

import concourse.bass as bass
import concourse.mybir as mybir
import concourse.tile as tile
from concourse import bacc, library_config
from concourse.bass_utils import run_bass_kernel_spmd

B, T_FULL, C = 4, 2048, 768
NH, HD = 12, 64
HL = NH // 2            # heads per core
NPAIR = HL // 2         # head pairs per core
NQK = HL * HD           # 384 features per core for each of q/k/v
N_CORES = 8
P = 128
SW = 512                # qt strip width
NC_T = C // P           # 6 contraction tiles
F32 = mybir.dt.float32
BF16 = mybir.dt.bfloat16
NPF = np.float32
NPBF = ml_dtypes.bfloat16

_CACHE: dict = {}


def build(T: int = T_FULL):
    NT = T // P
    NSTRIP = T // SW
    nc = bacc.Bacc("TRN2", target_bir_lowering=False, debug=False,
                   num_devices=N_CORES)
    x_d = nc.dram_tensor("x", [T, C], BF16, kind="ExternalInput")
    w_d = nc.dram_tensor("wqkv", [C, 3 * NQK], BF16, kind="ExternalInput")
    bqk_d = nc.dram_tensor("bqk", [P, 2 * NPAIR], F32, kind="ExternalInput")
    bv_d = nc.dram_tensor("bv", [1, NQK], F32, kind="ExternalInput")
    wp_d = nc.dram_tensor("wp", [NQK, C], BF16, kind="ExternalInput")
    tri_d = nc.dram_tensor("tri", [P, P], F32, kind="ExternalInput")
    out_d = nc.dram_tensor("out", [T, C], F32, kind="ExternalOutput")

    EXP = mybir.ActivationFunctionType.Exp
    ADD = mybir.AluOpType.add
    MUL = mybir.AluOpType.mult

    with ExitStack() as ctx:
        tc = ctx.enter_context(tile.TileContext(nc))
        persist = ctx.enter_context(tc.tile_pool(name="persist", bufs=1))
        ppool = ctx.enter_context(tc.tile_pool(name="pt", bufs=6))
        smallp = ctx.enter_context(tc.tile_pool(name="small", bufs=4))
        outp = ctx.enter_context(tc.tile_pool(name="outsb", bufs=3))
        ps_s = ctx.enter_context(tc.tile_pool(name="ps_s", bufs=2, space="PSUM"))
        ps_av = ctx.enter_context(tc.tile_pool(name="ps_av", bufs=3, space="PSUM"))

        nc.gpsimd.load_library(library_config.attn)

        # ---- persistent inputs (spread across the two DMA-capable queues) ----
        w_sb = persist.tile([P, NC_T, 3 * NQK], BF16)
        nc.scalar.dma_start(w_sb[:], w_d.ap().rearrange("(a p) n -> p a n", p=P))
        xT = persist.tile([P, NC_T, T], BF16)
        for cb in range(NC_T):
            eng = nc.sync if cb % 2 == 0 else nc.scalar
            eng.dma_start_transpose(xT[:, cb, :],
                                    x_d.ap()[:, cb * P:(cb + 1) * P])
        bqk_sb = persist.tile([P, 2 * NPAIR], F32)
        nc.sync.dma_start(bqk_sb[:], bqk_d.ap())
        bv_bc = persist.tile([P, NQK], F32)
        bv_ap = bv_d.ap()
        bv_bcast = bass.AP(tensor=bv_ap.tensor, offset=bv_ap.offset,
                           ap=[[0, P], [1, NQK]])
        nc.scalar.dma_start(bv_bc[:], bv_bcast)
        tri_sb = persist.tile([P, P], F32)
        nc.sync.dma_start(tri_sb[:], tri_d.ap())
        wp_sb = persist.tile([P, NQK // P, C], BF16)
        nc.sync.dma_start(wp_sb[:], wp_d.ap().rearrange("(a p) n -> p a n", p=P))

        qT = persist.tile([P, NPAIR, T], BF16)
        kT = persist.tile([P, NPAIR, T], BF16)
        # v with a ones column per head: [kt, tt, head, 0:64]=v, [..., 64]=1
        v_sb = persist.tile([P, NT, HL, HD + 1], BF16)
        nc.vector.memset(v_sb[:, :, :, HD:HD + 1], 1.0)
        aoT = persist.tile([P, NPAIR, T], BF16)

        for p in range(NPAIR):
            # ---- QKV for this pair ----
            for which, dst in ((p, qT), (NPAIR + p, kT)):
                for s in range(NSTRIP):
                    ps_t = ps_s.tile([P, SW], F32, tag="s")
                    for cb in range(NC_T):
                        nc.tensor.matmul(
                            ps_t[:], w_sb[:, cb, which * P:(which + 1) * P],
                            xT[:, cb, s * SW:(s + 1) * SW],
                            start=(cb == 0), stop=(cb == NC_T - 1))
                    nc.vector.tensor_scalar_add(
                        dst[:, p, s * SW:(s + 1) * SW], ps_t[:],
                        bqk_sb[:, which:which + 1])
            for tt in range(NT):
                ps_t = ps_s.tile([P, SW], F32, tag="s")
                vcols = 2 * NQK + p * P
                for cb in range(NC_T):
                    nc.tensor.matmul(
                        ps_t[:, 0:P], xT[:, cb, tt * P:(tt + 1) * P],
                        w_sb[:, cb, vcols:vcols + P],
                        start=(cb == 0), stop=(cb == NC_T - 1))
                nc.vector.tensor_tensor(
                    out=v_sb[:, tt, 2 * p:2 * p + 2, 0:HD], in0=ps_t[:, 0:P],
                    in1=bv_bc[:, p * P:(p + 1) * P], op=ADD)

            # ---- attention for this pair ----
            hA, hB = 2 * p, 2 * p + 1
            for s in range(NSTRIP):
                n_kt = 4 * (s + 1)
                avA = ps_av.tile([P, SW], F32, tag="av")
                avB = ps_av.tile([P, SW], F32, tag="av")
                for g in range(n_kt // 2):
                    sA = ps_s.tile([P, 2, SW], F32, tag="s")
                    sB = ps_s.tile([P, 2, SW], F32, tag="s")
                    for sub in range(2):
                        kt = 2 * g + sub
                        j = kt - 4 * s
                        c0 = max(j, 0) * P
                        kts = slice(kt * P, (kt + 1) * P)
                        qts = slice(s * SW + c0, (s + 1) * SW)
                        nc.tensor.matmul(sA[:, sub, c0:SW], kT[0:64, p, kts],
                                         qT[0:64, p, qts], start=True, stop=True)
                        nc.tensor.matmul(sB[:, sub, c0:SW], kT[64:P, p, kts],
                                         qT[64:P, p, qts], start=True, stop=True)
                        if j >= 0:  # diagonal block: -1e9 tri mask pre-exp
                            for st in (sA, sB):
                                nc.vector.tensor_tensor(
                                    out=st[:, sub, c0:c0 + P],
                                    in0=st[:, sub, c0:c0 + P],
                                    in1=tri_sb[:], op=ADD)
                    pA = ppool.tile([P, 2, SW], BF16, tag="pt")
                    pB = ppool.tile([P, 2, SW], BF16, tag="pt")
                    nc.scalar.activation(pA[:], sA[:], EXP)
                    nc.scalar.activation(pB[:], sB[:], EXP)
                    for sub in range(2):
                        kt = 2 * g + sub
                        c0 = max(kt - 4 * s, 0) * P
                        first, last = kt == 0, kt == n_kt - 1
                        nc.tensor.matmul(
                            avA[0:HD + 1, c0:SW], v_sb[:, kt, hA, :],
                            pA[:, sub, c0:SW], start=first, stop=last,
                            skip_group_check=True)
                        nc.tensor.matmul(
                            avB[0:HD + 1, c0:SW], v_sb[:, kt, hB, :],
                            pB[:, sub, c0:SW], start=first, stop=last,
                            skip_group_check=True)
                # normalize: rl = 1/l (l = row 64 of av psum), broadcast, mul
                # (reciprocal_approx_fast misreads PSUM input - bounce via SBUF)
                lA = smallp.tile([1, SW], F32, tag="lrow")
                lB = smallp.tile([1, SW], F32, tag="lrow")
                nc.vector.tensor_copy(lA[:], avA[HD:HD + 1, :])
                nc.vector.tensor_copy(lB[:], avB[HD:HD + 1, :])
                rlA = smallp.tile([1, SW], F32, tag="rl")
                rlB = smallp.tile([1, SW], F32, tag="rl")
                nc.vector.reciprocal_approx_fast(rlA[:], lA[:])
                nc.vector.reciprocal_approx_fast(rlB[:], lB[:])
                rbA = smallp.tile([HD, SW], F32, tag="rb")
                rbB = smallp.tile([HD, SW], F32, tag="rb")
                nc.gpsimd.partition_broadcast(rbA[:], rlA[:], channels=HD)
                nc.gpsimd.partition_broadcast(rbB[:], rlB[:], channels=HD)
                ss = slice(s * SW, (s + 1) * SW)
                nc.vector.tensor_tensor(out=aoT[0:HD, p, ss],
                                        in0=avA[0:HD, :], in1=rbA[:], op=MUL)
                nc.vector.tensor_tensor(out=aoT[HD:P, p, ss],
                                        in0=avB[0:HD, :], in1=rbB[:], op=MUL)

        # ---- output projection (partial: this core's feature rows) ----
        for tt in range(NT):
            pr = ps_s.tile([P, 2, SW], F32, tag="s")
            tts = slice(tt * P, (tt + 1) * P)
            for ft in range(NQK // P):
                nc.tensor.matmul(pr[:, 0, :], aoT[:, ft, tts],
                                 wp_sb[:, ft, 0:SW],
                                 start=(ft == 0), stop=(ft == NQK // P - 1))
                nc.tensor.matmul(pr[:, 1, 0:C - SW], aoT[:, ft, tts],
                                 wp_sb[:, ft, SW:C],
                                 start=(ft == 0), stop=(ft == NQK // P - 1))
            ot = outp.tile([P, C], F32)
            nc.vector.tensor_copy(ot[:, 0:SW], pr[:, 0, :])
            nc.vector.tensor_copy(ot[:, SW:C], pr[:, 1, 0:C - SW])
            nc.sync.dma_start(out_d.ap()[tts, :], ot[:])

    nc.compile()
    return nc


def make_in_maps(x, w_attn, b_attn, w_proj):
    """Shard the full inputs into per-core input maps (host side)."""
    scale = 1.0 / math.sqrt(HD)
    tri = np.where(np.arange(P)[:, None] <= np.arange(P)[None, :],
                   0.0, -1e9).astype(NPF)
    in_maps = []
    for core in range(N_CORES):
        b, g = divmod(core, 2)
        cs = slice(g * NQK, (g + 1) * NQK)
        wq = w_attn[:, 0 * C:1 * C][:, cs] * scale
        wk = w_attn[:, 1 * C:2 * C][:, cs]
        wv = w_attn[:, 2 * C:3 * C][:, cs]
        wqkv = np.concatenate([wq, wk, wv], axis=1).astype(NPBF)
        bq = b_attn[0 * C:1 * C][cs] * scale
        bk = b_attn[1 * C:2 * C][cs]
        bqk = np.ascontiguousarray(
            np.concatenate([bq, bk]).reshape(2 * NPAIR, P).T).astype(NPF)
        bv = b_attn[2 * C:3 * C][cs].astype(NPF).reshape(1, NQK)
        wp = w_proj[g * NQK:(g + 1) * NQK, :].astype(NPBF)
        in_maps.append({
            "x": np.ascontiguousarray(x[b]).astype(NPBF),
            "wqkv": wqkv, "bqk": bqk, "bv": bv, "wp": wp, "tri": tri,
        })
    return in_maps


def combine_outputs(results, b_proj):
    outs = [results[i]["out"] for i in range(N_CORES)]
    out = np.stack([outs[2 * b] + outs[2 * b + 1] for b in range(B)])
    return (out + b_proj[None, None, :].astype(NPF)).astype(NPF)


def kernel(x, w_attn, b_attn, w_proj, b_proj):
    x = np.asarray(x, dtype=NPF)
    w_attn = np.asarray(w_attn, dtype=NPF)
    b_attn = np.asarray(b_attn, dtype=NPF)
    w_proj = np.asarray(w_proj, dtype=NPF)
    b_proj = np.asarray(b_proj, dtype=NPF)
    if "nc" not in _CACHE:
        _CACHE["nc"] = build(T_FULL)
    nc = _CACHE["nc"]
    in_maps = make_in_maps(x, w_attn, b_attn, w_proj)
    res = run_bass_kernel_spmd(nc, in_maps, list(range(N_CORES)))
    return combine_outputs(res.results, b_proj)


# revision 21
# speedup vs baseline: 1.3636x; 1.2643x over previous
"""Causal multi-head attention block (QKV proj -> attention -> out proj) on 8
Trainium2 NeuronCores.

Sharding: core i handles batch b = i//2 and head-group g = i%2 (6 of 12 heads).
Each core computes its heads' attention output and a partial output projection
(rows g*384:(g+1)*384 of w_proj); the host sums the two partials per batch and
adds b_proj.

On-core dataflow (per core):
  x^T tiles  [c,t]   via DMA-transpose (bf16)
  q^T, k^T   [n,t]   = w-stationary matmuls, pair-stacked 2 heads/tile
  v          [t,n]   natural layout, with a ones column appended per head
  S^T        [kt,qt] = k^T-stationary matmul, row-packed pairs (K=64 halves)
  P^T        = exp(S^T)  (no max subtraction: |scores| is O(10), safe in fp32)
  [out^T; l] [65,qt] = [v|1]-stationary matmul per head (l = softmax denom)
  out^T * (1/l)      -> ao^T [f,t], fed as lhsT to the projection matmul
Causal masking: only kt-tiles <= qt are computed; diagonal 128x128 blocks get a
0/1 triangular mask multiplied into P^T post-exp; fully-masked column ranges
are skipped via partial-free-dim matmuls into the same PSUM bank.

The emission order interleaves pair p+1's QKV matmul groups (and the final
projection) into pair p's attention stream: the attention phase is
ScalarE(exp)-bound, and PE executes in program order, so without interleaving
the PE sits idle between score/AV bursts, HAM re-throttles the clock to
1.2 GHz, and every matmul doubles in cost.
"""

import math
from contextlib import ExitStack

import numpy as np
import ml_dtypes

import concourse.bass as bass
import concourse.mybir as mybir
import concourse.tile as tile
from concourse import bacc, library_config
from concourse.bass_utils import run_bass_kernel_spmd

B, T_FULL, C = 4, 2048, 768
NH, HD = 12, 64
HL = NH // 2            # heads per core
NPAIR = HL // 2         # head pairs per core
NQK = HL * HD           # 384 features per core for each of q/k/v
N_CORES = 8
P = 128
SW = 512                # qt strip width
NC_T = C // P           # 6 contraction tiles
F32 = mybir.dt.float32
BF16 = mybir.dt.bfloat16
NPF = np.float32
NPBF = ml_dtypes.bfloat16

_CACHE: dict = {}


def build(T: int = T_FULL, interleave_on: bool = True):
    NT = T // P
    NSTRIP = T // SW
    nc = bacc.Bacc("TRN2", target_bir_lowering=False, debug=False,
                   num_devices=N_CORES)
    x_d = nc.dram_tensor("x", [T, C], BF16, kind="ExternalInput")
    w_d = nc.dram_tensor("wqkv", [C, 3 * NQK], BF16, kind="ExternalInput")
    bqk_d = nc.dram_tensor("bqk", [P, 2 * NPAIR], F32, kind="ExternalInput")
    bv_d = nc.dram_tensor("bv", [1, NQK], F32, kind="ExternalInput")
    wp_d = nc.dram_tensor("wp", [NQK, C], BF16, kind="ExternalInput")
    tri_d = nc.dram_tensor("tri", [P, P], F32, kind="ExternalInput")
    out_d = nc.dram_tensor("out", [T, C], F32, kind="ExternalOutput")

    EXP = mybir.ActivationFunctionType.Exp
    ADD = mybir.AluOpType.add
    MUL = mybir.AluOpType.mult

    with ExitStack() as ctx:
        tc = ctx.enter_context(tile.TileContext(nc))
        persist = ctx.enter_context(tc.tile_pool(name="persist", bufs=1))
        ppool = ctx.enter_context(tc.tile_pool(name="pt", bufs=6))
        smallp = ctx.enter_context(tc.tile_pool(name="small", bufs=4))
        outp = ctx.enter_context(tc.tile_pool(name="outsb", bufs=3))
        ps_s = ctx.enter_context(tc.tile_pool(name="ps_s", bufs=2, space="PSUM"))
        ps_q = ctx.enter_context(tc.tile_pool(name="ps_q", bufs=2, space="PSUM"))
        ps_av = ctx.enter_context(tc.tile_pool(name="ps_av", bufs=2, space="PSUM"))

        nc.gpsimd.load_library(library_config.attn)

        # ---- persistent inputs (spread across the two DMA-capable queues) ----
        w_sb = persist.tile([P, NC_T, 3 * NQK], BF16)
        nc.scalar.dma_start(w_sb[:], w_d.ap().rearrange("(a p) n -> p a n", p=P))
        tri_sb = persist.tile([P, P], F32)
        nc.sync.dma_start(tri_sb[:], tri_d.ap())
        bqk_sb = persist.tile([P, 2 * NPAIR], F32)
        nc.sync.dma_start(bqk_sb[:], bqk_d.ap())
        bv_bc = persist.tile([P, NQK], F32)
        bv_ap = bv_d.ap()
        bv_bcast = bass.AP(tensor=bv_ap.tensor, offset=bv_ap.offset,
                           ap=[[0, P], [1, NQK]])
        nc.sync.dma_start(bv_bc[:], bv_bcast)
        # x^T whole-column transposes (chunked slices raced with consumers)
        xT = persist.tile([P, NC_T, T], BF16)
        for cb in range(NC_T):
            eng = nc.sync if cb % 2 == 0 else nc.scalar
            eng.dma_start_transpose(xT[:, cb, :],
                                    x_d.ap()[:, cb * P:(cb + 1) * P])
        wp_sb = persist.tile([P, NQK // P, C], BF16)
        nc.sync.dma_start(wp_sb[:], wp_d.ap().rearrange("(a p) n -> p a n", p=P))

        # HAM warmup: keep PE busy with throwaway matmuls while x^T streams in
        warm_ps = ps_q.tile([P, P], F32, tag="q")
        for i in range(28):
            nc.tensor.matmul(warm_ps[:], tri_sb[:], tri_sb[:],
                             start=(i == 0), stop=(i == 27),
                             skip_group_check=True)

        # per-pair tensors (separate tiles so pair p+1 writes don't create
        # false deps against pair p reads)
        qT = [persist.tile([P, T], BF16, name=f"qT{i}", tag=f"qT{i}") for i in range(NPAIR)]
        kT = [persist.tile([P, T], BF16, name=f"kT{i}", tag=f"kT{i}") for i in range(NPAIR)]
        v_sb = [persist.tile([P, NT, 2, HD + 1], BF16, name=f"v{i}", tag=f"v{i}")
                for i in range(NPAIR)]
        aoT = [persist.tile([P, T], BF16, name=f"aoT{i}", tag=f"aoT{i}") for i in range(NPAIR)]
        for p in range(NPAIR):
            nc.vector.memset(v_sb[p][:, :, :, HD:HD + 1], 1.0)

        def qk_unit(p, is_k, s):
            dst = kT[p] if is_k else qT[p]
            # feature columns in w_sb: q pairs 0..2 then k pairs 0..2
            bidx = NPAIR + p if is_k else p
            fi = bidx * P
            ps_t = ps_q.tile([P, SW], F32, tag="q")
            for cb in range(NC_T):
                nc.tensor.matmul(
                    ps_t[:], w_sb[:, cb, fi:fi + P],
                    xT[:, cb, s * SW:(s + 1) * SW],
                    start=(cb == 0), stop=(cb == NC_T - 1))
            nc.vector.tensor_scalar_add(
                dst[:, s * SW:(s + 1) * SW], ps_t[:], bqk_sb[:, bidx:bidx + 1])

        def v_unit(p, tt):
            ps_t = ps_q.tile([P, SW], F32, tag="q")
            vcols = 2 * NQK + p * P
            for cb in range(NC_T):
                nc.tensor.matmul(
                    ps_t[:, 0:P], xT[:, cb, tt * P:(tt + 1) * P],
                    w_sb[:, cb, vcols:vcols + P],
                    start=(cb == 0), stop=(cb == NC_T - 1))
            nc.vector.tensor_tensor(
                out=v_sb[p][:, tt, :, 0:HD], in0=ps_t[:, 0:P],
                in1=bv_bc[:, p * P:(p + 1) * P], op=ADD)

        def qkv_units(p):
            us = []
            for is_k in (False, True):
                for s in range(NSTRIP):
                    us.append(lambda k=is_k, s=s: qk_unit(p, k, s))
            for tt in range(NT):
                us.append(lambda tt=tt: v_unit(p, tt))
            return us

        def proj_unit(tt):
            tts = slice(tt * P, (tt + 1) * P)
            ot = outp.tile([P, C], F32)
            for nch, n0, n1 in ((0, 0, SW), (1, SW, C)):
                pr = ps_q.tile([P, SW], F32, tag="q")
                for ft in range(NQK // P):
                    nc.tensor.matmul(pr[:, 0:n1 - n0], aoT[ft][:, tts],
                                     wp_sb[:, ft, n0:n1],
                                     start=(ft == 0), stop=(ft == NQK // P - 1))
                nc.vector.tensor_copy(ot[:, n0:n1], pr[:, 0:n1 - n0])
            nc.sync.dma_start(out_d.ap()[tts, :], ot[:])

        def attn_units(p):
            us = []
            state = {}

            def group_unit(s, g, n_kt):
                def run():
                    if g == 0:
                        state["avA"] = ps_av.tile([P, SW], F32, name="avA", tag="av")
                        state["avB"] = ps_av.tile([P, SW], F32, name="avB", tag="av")
                    avA, avB = state["avA"], state["avB"]
                    sA = ps_s.tile([P, 2, SW], F32, tag="s")
                    sB = ps_s.tile([P, 2, SW], F32, tag="s")
                    for sub in range(2):
                        kt = 2 * g + sub
                        j = kt - 4 * s
                        c0 = max(j, 0) * P
                        kts = slice(kt * P, (kt + 1) * P)
                        qts = slice(s * SW + c0, (s + 1) * SW)
                        nc.tensor.matmul(sA[:, sub, c0:SW], kT[p][0:64, kts],
                                         qT[p][0:64, qts], start=True,
                                         stop=True)
                        nc.tensor.matmul(sB[:, sub, c0:SW], kT[p][64:P, kts],
                                         qT[p][64:P, qts], start=True,
                                         stop=True)
                        if j >= 0:  # diagonal: -1e9 tri mask pre-exp
                            for st in (sA, sB):
                                nc.vector.tensor_tensor(
                                    out=st[:, sub, c0:c0 + P],
                                    in0=st[:, sub, c0:c0 + P],
                                    in1=tri_sb[:], op=ADD)
                    pA = ppool.tile([P, 2, SW], BF16, tag="pt")
                    pB = ppool.tile([P, 2, SW], BF16, tag="pt")
                    nc.scalar.activation(pA[:], sA[:], EXP)
                    nc.scalar.activation(pB[:], sB[:], EXP)
                    for sub in range(2):
                        kt = 2 * g + sub
                        c0 = max(kt - 4 * s, 0) * P
                        first, last = kt == 0, kt == n_kt - 1
                        nc.tensor.matmul(
                            avA[0:HD + 1, c0:SW], v_sb[p][:, kt, 0, :],
                            pA[:, sub, c0:SW], start=first, stop=last,
                            skip_group_check=True)
                        nc.tensor.matmul(
                            avB[0:HD + 1, c0:SW], v_sb[p][:, kt, 1, :],
                            pB[:, sub, c0:SW], start=first, stop=last,
                            skip_group_check=True)
                return run

            def norm_unit(s):
                def run():
                    avA, avB = state["avA"], state["avB"]
                    lA = smallp.tile([1, SW], F32, tag="lrow")
                    lB = smallp.tile([1, SW], F32, tag="lrow")
                    nc.vector.tensor_copy(lA[:], avA[HD:HD + 1, :])
                    nc.vector.tensor_copy(lB[:], avB[HD:HD + 1, :])
                    rlA = smallp.tile([1, SW], F32, tag="rl")
                    rlB = smallp.tile([1, SW], F32, tag="rl")
                    nc.vector.reciprocal_approx_fast(rlA[:], lA[:])
                    nc.vector.reciprocal_approx_fast(rlB[:], lB[:])
                    rbA = smallp.tile([HD, SW], F32, tag="rb")
                    rbB = smallp.tile([HD, SW], F32, tag="rb")
                    nc.gpsimd.partition_broadcast(rbA[:], rlA[:], channels=HD)
                    nc.gpsimd.partition_broadcast(rbB[:], rlB[:], channels=HD)
                    ss = slice(s * SW, (s + 1) * SW)
                    nc.vector.tensor_tensor(out=aoT[p][0:HD, ss],
                                            in0=avA[0:HD, :], in1=rbA[:],
                                            op=MUL)
                    nc.vector.tensor_tensor(out=aoT[p][HD:P, ss],
                                            in0=avB[0:HD, :], in1=rbB[:],
                                            op=MUL)
                return run

            for s in range(NSTRIP):
                n_kt = 4 * (s + 1)
                for g in range(n_kt // 2):
                    us.append(group_unit(s, g, n_kt))
                us.append(norm_unit(s))
            return us

        def interleave(a_units, b_units):
            """Emit a_units (attention, ACT-heavy) with b_units (PE-heavy)
            spread evenly between them. b_units are (min_a_idx, fn): fn may
            only be emitted after a_units[min_a_idx - 1] (data dependency)."""
            if not a_units:
                for _, u in b_units:
                    u()
                return
            ratio = len(b_units) / len(a_units)
            bi = 0
            for i, u in enumerate(a_units):
                u()
                target = int(round((i + 1) * ratio))
                while bi < len(b_units) and bi < target \
                        and b_units[bi][0] <= i + 1:
                    b_units[bi][1]()
                    bi += 1
            while bi < len(b_units):
                b_units[bi][1]()
                bi += 1

        # pair 0 QKV up front
        for u in qkv_units(0):
            u()
        # attention(p) unit index right after which norm(s) has been emitted:
        # strips contribute 2(s+1) group units + 1 norm unit each
        norm_done = {}
        acc = 0
        for s in range(NSTRIP):
            acc += 2 * (s + 1) + 1
            norm_done[s] = acc
        for p in range(NPAIR):
            if p + 1 < NPAIR:
                fill = [(0, u) for u in qkv_units(p + 1)]
            else:
                fill = [(norm_done[min(tt // 4, NSTRIP - 1)],
                         lambda tt=tt: proj_unit(tt)) for tt in range(NT)]
            if interleave_on:
                interleave(attn_units(p), fill)
            else:
                for u in attn_units(p):
                    u()
                for _, u in fill:
                    u()

    nc.compile()
    return nc


def make_in_maps(x, w_attn, b_attn, w_proj):
    """Shard the full inputs into per-core input maps (host side)."""
    scale = 1.0 / math.sqrt(HD)
    tri = np.where(np.arange(P)[:, None] <= np.arange(P)[None, :],
                   0.0, -1e9).astype(NPF)
    in_maps = []
    for core in range(N_CORES):
        b, g = divmod(core, 2)
        cs = slice(g * NQK, (g + 1) * NQK)
        wq = w_attn[:, 0 * C:1 * C][:, cs] * scale
        wk = w_attn[:, 1 * C:2 * C][:, cs]
        wv = w_attn[:, 2 * C:3 * C][:, cs]
        wqkv = np.concatenate([wq, wk, wv], axis=1).astype(NPBF)
        bq = b_attn[0 * C:1 * C][cs] * scale
        bk = b_attn[1 * C:2 * C][cs]
        bqk = np.ascontiguousarray(
            np.concatenate([bq, bk]).reshape(2 * NPAIR, P).T).astype(NPF)
        bv = b_attn[2 * C:3 * C][cs].astype(NPF).reshape(1, NQK)
        wp = w_proj[g * NQK:(g + 1) * NQK, :].astype(NPBF)
        in_maps.append({
            "x": np.ascontiguousarray(x[b]).astype(NPBF),
            "wqkv": wqkv, "bqk": bqk, "bv": bv, "wp": wp, "tri": tri,
        })
    return in_maps


def combine_outputs(results, b_proj):
    outs = [results[i]["out"] for i in range(N_CORES)]
    out = np.stack([outs[2 * b] + outs[2 * b + 1] for b in range(B)])
    return (out + b_proj[None, None, :].astype(NPF)).astype(NPF)


def kernel(x, w_attn, b_attn, w_proj, b_proj):
    x = np.asarray(x, dtype=NPF)
    w_attn = np.asarray(w_attn, dtype=NPF)
    b_attn = np.asarray(b_attn, dtype=NPF)
    w_proj = np.asarray(w_proj, dtype=NPF)
    b_proj = np.asarray(b_proj, dtype=NPF)
    if "nc" not in _CACHE:
        _CACHE["nc"] = build(T_FULL)
    nc = _CACHE["nc"]
    in_maps = make_in_maps(x, w_attn, b_attn, w_proj)
    res = run_bass_kernel_spmd(nc, in_maps, list(range(N_CORES)))
    return combine_outputs(res.results, b_proj)


# revision 23
# speedup vs baseline: 1.3669x; 1.0024x over previous
"""Causal multi-head attention block (QKV proj -> attention -> out proj) on 8
Trainium2 NeuronCores.

Sharding: core i handles batch b = i//2 and head-group g = i%2 (6 of 12 heads).
Each core computes its heads' attention output and a partial output projection
(rows g*384:(g+1)*384 of w_proj); the host sums the two partials per batch and
adds b_proj.

On-core dataflow (per core):
  x^T tiles  [c,t]   via DMA-transpose (bf16)
  q^T, k^T   [n,t]   = w-stationary matmuls, pair-stacked 2 heads/tile
  v          [t,n]   natural layout, with a ones column appended per head
  S^T        [kt,qt] = k^T-stationary matmul, row-packed pairs (K=64 halves)
  P^T        = exp(S^T)  (no max subtraction: |scores| is O(10), safe in fp32)
  [out^T; l] [65,qt] = [v|1]-stationary matmul per head (l = softmax denom)
  out^T * (1/l)      -> ao^T [f,t], fed as lhsT to the projection matmul
Causal masking: only kt-tiles <= qt are computed; diagonal 128x128 blocks get a
0/1 triangular mask multiplied into P^T post-exp; fully-masked column ranges
are skipped via partial-free-dim matmuls into the same PSUM bank.

The emission order interleaves pair p+1's QKV matmul groups (and the final
projection) into pair p's attention stream: the attention phase is
ScalarE(exp)-bound, and PE executes in program order, so without interleaving
the PE sits idle between score/AV bursts, HAM re-throttles the clock to
1.2 GHz, and every matmul doubles in cost.
"""

import math
from contextlib import ExitStack

import numpy as np
import ml_dtypes

import concourse.bass as bass
import concourse.mybir as mybir
import concourse.tile as tile
from concourse import bacc, library_config
from concourse.bass_utils import run_bass_kernel_spmd

B, T_FULL, C = 4, 2048, 768
NH, HD = 12, 64
HL = NH // 2            # heads per core
NPAIR = HL // 2         # head pairs per core
NQK = HL * HD           # 384 features per core for each of q/k/v
N_CORES = 8
P = 128
SW = 512                # qt strip width
NC_T = C // P           # 6 contraction tiles
F32 = mybir.dt.float32
BF16 = mybir.dt.bfloat16
NPF = np.float32
NPBF = ml_dtypes.bfloat16

_CACHE: dict = {}


def build(T: int = T_FULL, interleave_on: bool = True):
    NT = T // P
    NSTRIP = T // SW
    nc = bacc.Bacc("TRN2", target_bir_lowering=False, debug=False,
                   num_devices=N_CORES)
    x_d = nc.dram_tensor("x", [T, C], BF16, kind="ExternalInput")
    w_d = nc.dram_tensor("wqkv", [C, 3 * NQK], BF16, kind="ExternalInput")
    bqk_d = nc.dram_tensor("bqk", [P, 2 * NPAIR], F32, kind="ExternalInput")
    bv_d = nc.dram_tensor("bv", [1, NQK], F32, kind="ExternalInput")
    wp_d = nc.dram_tensor("wp", [NQK, C], BF16, kind="ExternalInput")
    tri_d = nc.dram_tensor("tri", [P, P], F32, kind="ExternalInput")
    out_d = nc.dram_tensor("out", [T, C], F32, kind="ExternalOutput")

    EXP = mybir.ActivationFunctionType.Exp
    ADD = mybir.AluOpType.add
    MUL = mybir.AluOpType.mult

    with ExitStack() as ctx:
        tc = ctx.enter_context(tile.TileContext(nc))
        persist = ctx.enter_context(tc.tile_pool(name="persist", bufs=1))
        ppool = ctx.enter_context(tc.tile_pool(name="pt", bufs=6))
        smallp = ctx.enter_context(tc.tile_pool(name="small", bufs=4))
        outp = ctx.enter_context(tc.tile_pool(name="outsb", bufs=3))
        ps_s = ctx.enter_context(tc.tile_pool(name="ps_s", bufs=2, space="PSUM"))
        ps_q = ctx.enter_context(tc.tile_pool(name="ps_q", bufs=2, space="PSUM"))
        ps_av = ctx.enter_context(tc.tile_pool(name="ps_av", bufs=2, space="PSUM"))

        nc.gpsimd.load_library(library_config.attn)

        # ---- persistent inputs (spread across the two DMA-capable queues) ----
        w_sb = persist.tile([P, NC_T, 3 * NQK], BF16)
        nc.scalar.dma_start(w_sb[:], w_d.ap().rearrange("(a p) n -> p a n", p=P))
        tri_sb = persist.tile([P, P], F32)
        nc.sync.dma_start(tri_sb[:], tri_d.ap())
        bqk_sb = persist.tile([P, 2 * NPAIR], F32)
        nc.sync.dma_start(bqk_sb[:], bqk_d.ap())
        bv_bc = persist.tile([P, NQK], F32)
        bv_ap = bv_d.ap()
        bv_bcast = bass.AP(tensor=bv_ap.tensor, offset=bv_ap.offset,
                           ap=[[0, P], [1, NQK]])
        nc.sync.dma_start(bv_bc[:], bv_bcast)
        # x^T whole-column transposes (chunked slices raced with consumers)
        xT = persist.tile([P, NC_T, T], BF16)
        for cb in range(NC_T):
            eng = nc.sync if cb % 2 == 0 else nc.scalar
            eng.dma_start_transpose(xT[:, cb, :],
                                    x_d.ap()[:, cb * P:(cb + 1) * P])
        wp_sb = persist.tile([P, NQK // P, C], BF16)
        nc.sync.dma_start(wp_sb[:], wp_d.ap().rearrange("(a p) n -> p a n", p=P))

        # HAM warmup: keep PE busy with throwaway matmuls while x^T streams in
        warm_ps = ps_q.tile([P, P], F32, tag="q")
        for i in range(60):
            nc.tensor.matmul(warm_ps[:], tri_sb[:], tri_sb[:],
                             start=(i == 0), stop=(i == 59),
                             skip_group_check=True)

        # per-pair tensors (separate tiles so pair p+1 writes don't create
        # false deps against pair p reads)
        qT = [persist.tile([P, T], BF16, name=f"qT{i}", tag=f"qT{i}") for i in range(NPAIR)]
        kT = [persist.tile([P, T], BF16, name=f"kT{i}", tag=f"kT{i}") for i in range(NPAIR)]
        v_sb = [persist.tile([P, NT, 2, HD + 1], BF16, name=f"v{i}", tag=f"v{i}")
                for i in range(NPAIR)]
        aoT = [persist.tile([P, T], BF16, name=f"aoT{i}", tag=f"aoT{i}") for i in range(NPAIR)]
        for p in range(NPAIR):
            nc.vector.memset(v_sb[p][:, :, :, HD:HD + 1], 1.0)

        def qk_unit(p, is_k, s):
            dst = kT[p] if is_k else qT[p]
            # feature columns in w_sb: q pairs 0..2 then k pairs 0..2
            bidx = NPAIR + p if is_k else p
            fi = bidx * P
            ps_t = ps_q.tile([P, SW], F32, tag="q")
            for cb in range(NC_T):
                nc.tensor.matmul(
                    ps_t[:], w_sb[:, cb, fi:fi + P],
                    xT[:, cb, s * SW:(s + 1) * SW],
                    start=(cb == 0), stop=(cb == NC_T - 1))
            nc.vector.tensor_scalar_add(
                dst[:, s * SW:(s + 1) * SW], ps_t[:], bqk_sb[:, bidx:bidx + 1])

        def v_unit(p, tt):
            ps_t = ps_q.tile([P, SW], F32, tag="q")
            vcols = 2 * NQK + p * P
            for cb in range(NC_T):
                nc.tensor.matmul(
                    ps_t[:, 0:P], xT[:, cb, tt * P:(tt + 1) * P],
                    w_sb[:, cb, vcols:vcols + P],
                    start=(cb == 0), stop=(cb == NC_T - 1))
            nc.vector.tensor_tensor(
                out=v_sb[p][:, tt, :, 0:HD], in0=ps_t[:, 0:P],
                in1=bv_bc[:, p * P:(p + 1) * P], op=ADD)

        def qkv_units(p):
            us = []
            for is_k in (False, True):
                for s in range(NSTRIP):
                    us.append(lambda k=is_k, s=s: qk_unit(p, k, s))
            for tt in range(NT):
                us.append(lambda tt=tt: v_unit(p, tt))
            return us

        def proj_unit(tt):
            tts = slice(tt * P, (tt + 1) * P)
            ot = outp.tile([P, C], F32)
            for nch, n0, n1 in ((0, 0, SW), (1, SW, C)):
                pr = ps_q.tile([P, SW], F32, tag="q")
                for ft in range(NQK // P):
                    nc.tensor.matmul(pr[:, 0:n1 - n0], aoT[ft][:, tts],
                                     wp_sb[:, ft, n0:n1],
                                     start=(ft == 0), stop=(ft == NQK // P - 1))
                nc.vector.tensor_copy(ot[:, n0:n1], pr[:, 0:n1 - n0])
            nc.sync.dma_start(out_d.ap()[tts, :], ot[:])

        def attn_units(p):
            us = []
            state = {}

            def group_unit(s, g, n_kt):
                def run():
                    if g == 0:
                        state["avA"] = ps_av.tile([P, SW], F32, name="avA", tag="av")
                        state["avB"] = ps_av.tile([P, SW], F32, name="avB", tag="av")
                    avA, avB = state["avA"], state["avB"]
                    sA = ps_s.tile([P, 2, SW], F32, tag="s")
                    sB = ps_s.tile([P, 2, SW], F32, tag="s")
                    for sub in range(2):
                        kt = 2 * g + sub
                        j = kt - 4 * s
                        c0 = max(j, 0) * P
                        kts = slice(kt * P, (kt + 1) * P)
                        qts = slice(s * SW + c0, (s + 1) * SW)
                        nc.tensor.matmul(sA[:, sub, c0:SW], kT[p][0:64, kts],
                                         qT[p][0:64, qts], start=True,
                                         stop=True)
                        nc.tensor.matmul(sB[:, sub, c0:SW], kT[p][64:P, kts],
                                         qT[p][64:P, qts], start=True,
                                         stop=True)
                        if j >= 0:  # diagonal: -1e9 tri mask pre-exp
                            for st in (sA, sB):
                                nc.vector.tensor_tensor(
                                    out=st[:, sub, c0:c0 + P],
                                    in0=st[:, sub, c0:c0 + P],
                                    in1=tri_sb[:], op=ADD)
                    pA = ppool.tile([P, 2, SW], BF16, tag="pt")
                    pB = ppool.tile([P, 2, SW], BF16, tag="pt")
                    nc.scalar.activation(pA[:], sA[:], EXP)
                    nc.scalar.activation(pB[:], sB[:], EXP)
                    for sub in range(2):
                        kt = 2 * g + sub
                        c0 = max(kt - 4 * s, 0) * P
                        first, last = kt == 0, kt == n_kt - 1
                        nc.tensor.matmul(
                            avA[0:HD + 1, c0:SW], v_sb[p][:, kt, 0, :],
                            pA[:, sub, c0:SW], start=first, stop=last,
                            skip_group_check=True)
                        nc.tensor.matmul(
                            avB[0:HD + 1, c0:SW], v_sb[p][:, kt, 1, :],
                            pB[:, sub, c0:SW], start=first, stop=last,
                            skip_group_check=True)
                return run

            def norm_unit(s):
                def run():
                    avA, avB = state["avA"], state["avB"]
                    lA = smallp.tile([1, SW], F32, tag="lrow")
                    lB = smallp.tile([1, SW], F32, tag="lrow")
                    nc.vector.tensor_copy(lA[:], avA[HD:HD + 1, :])
                    nc.vector.tensor_copy(lB[:], avB[HD:HD + 1, :])
                    rlA = smallp.tile([1, SW], F32, tag="rl")
                    rlB = smallp.tile([1, SW], F32, tag="rl")
                    nc.vector.reciprocal_approx_fast(rlA[:], lA[:])
                    nc.vector.reciprocal_approx_fast(rlB[:], lB[:])
                    rbA = smallp.tile([HD, SW], F32, tag="rb")
                    rbB = smallp.tile([HD, SW], F32, tag="rb")
                    nc.gpsimd.partition_broadcast(rbA[:], rlA[:], channels=HD)
                    nc.gpsimd.partition_broadcast(rbB[:], rlB[:], channels=HD)
                    ss = slice(s * SW, (s + 1) * SW)
                    nc.vector.tensor_tensor(out=aoT[p][0:HD, ss],
                                            in0=avA[0:HD, :], in1=rbA[:],
                                            op=MUL)
                    nc.vector.tensor_tensor(out=aoT[p][HD:P, ss],
                                            in0=avB[0:HD, :], in1=rbB[:],
                                            op=MUL)
                return run

            for s in range(NSTRIP):
                n_kt = 4 * (s + 1)
                for g in range(n_kt // 2):
                    us.append(group_unit(s, g, n_kt))
                us.append(norm_unit(s))
            return us

        def interleave(a_units, b_units):
            """Emit a_units (attention, ACT-heavy) with b_units (PE-heavy)
            spread evenly between them. b_units are (min_idx, deadline, fn):
            fn may only be emitted after a_units[min_idx - 1], and MUST be
            emitted before a_units[deadline] (prerequisite of that unit)."""
            if not a_units:
                for _, _, u in b_units:
                    u()
                return
            ratio = len(b_units) / len(a_units)
            bi = 0
            for i, u in enumerate(a_units):
                while bi < len(b_units) and b_units[bi][1] <= i:
                    b_units[bi][2]()
                    bi += 1
                u()
                target = int(round((i + 1) * ratio))
                while bi < len(b_units) and bi < target \
                        and b_units[bi][0] <= i + 1:
                    b_units[bi][2]()
                    bi += 1
            while bi < len(b_units):
                b_units[bi][2]()
                bi += 1

        # attention(p) a-unit bookkeeping: strips contribute 2(s+1) group
        # units + 1 norm unit each; cum[s] = first a-index AFTER strip s
        BIG = 10 ** 9
        cum = {}
        acc = 0
        for s in range(NSTRIP):
            acc += 2 * (s + 1) + 1
            cum[s] = acc
        norm_done = cum

        # minimal prefix of qkv(0) so attention(0) strip 0 can start
        qk_unit(0, False, 0)
        qk_unit(0, True, 0)
        for tt in range(min(4, NT)):
            v_unit(0, tt)
        # rest of qkv(0), deadline-gated: strip s+1 prereqs must be emitted
        # before attention(0) strip s+1 begins (a-index cum[s])
        fill0 = []
        for s in range(1, NSTRIP):
            dl = cum[s - 1]
            fill0.append((0, dl, lambda s=s: qk_unit(0, False, s)))
            fill0.append((0, dl, lambda s=s: qk_unit(0, True, s)))
            for tt in range(4 * s, min(4 * s + 4, NT)):
                fill0.append((0, dl, lambda tt=tt: v_unit(0, tt)))
        for p in range(NPAIR):
            fill = list(fill0) if p == 0 else []
            if p + 1 < NPAIR:
                fill += [(0, BIG, u) for u in qkv_units(p + 1)]
            else:
                fill += [(norm_done[min(tt // 4, NSTRIP - 1)], BIG,
                          lambda tt=tt: proj_unit(tt)) for tt in range(NT)]
            if interleave_on:
                interleave(attn_units(p), fill)
            else:
                for _, dl, u in fill:
                    if dl < BIG:
                        u()
                for u in attn_units(p):
                    u()
                for _, dl, u in fill:
                    if dl >= BIG:
                        u()

    nc.compile()
    return nc


def make_in_maps(x, w_attn, b_attn, w_proj):
    """Shard the full inputs into per-core input maps (host side)."""
    scale = 1.0 / math.sqrt(HD)
    tri = np.where(np.arange(P)[:, None] <= np.arange(P)[None, :],
                   0.0, -1e9).astype(NPF)
    in_maps = []
    for core in range(N_CORES):
        b, g = divmod(core, 2)
        cs = slice(g * NQK, (g + 1) * NQK)
        wq = w_attn[:, 0 * C:1 * C][:, cs] * scale
        wk = w_attn[:, 1 * C:2 * C][:, cs]
        wv = w_attn[:, 2 * C:3 * C][:, cs]
        wqkv = np.concatenate([wq, wk, wv], axis=1).astype(NPBF)
        bq = b_attn[0 * C:1 * C][cs] * scale
        bk = b_attn[1 * C:2 * C][cs]
        bqk = np.ascontiguousarray(
            np.concatenate([bq, bk]).reshape(2 * NPAIR, P).T).astype(NPF)
        bv = b_attn[2 * C:3 * C][cs].astype(NPF).reshape(1, NQK)
        wp = w_proj[g * NQK:(g + 1) * NQK, :].astype(NPBF)
        in_maps.append({
            "x": np.ascontiguousarray(x[b]).astype(NPBF),
            "wqkv": wqkv, "bqk": bqk, "bv": bv, "wp": wp, "tri": tri,
        })
    return in_maps


def combine_outputs(results, b_proj):
    outs = [results[i]["out"] for i in range(N_CORES)]
    out = np.stack([outs[2 * b] + outs[2 * b + 1] for b in range(B)])
    return (out + b_proj[None, None, :].astype(NPF)).astype(NPF)


def kernel(x, w_attn, b_attn, w_proj, b_proj):
    x = np.asarray(x, dtype=NPF)
    w_attn = np.asarray(w_attn, dtype=NPF)
    b_attn = np.asarray(b_attn, dtype=NPF)
    w_proj = np.asarray(w_proj, dtype=NPF)
    b_proj = np.asarray(b_proj, dtype=NPF)
    if "nc" not in _CACHE:
        _CACHE["nc"] = build(T_FULL)
    nc = _CACHE["nc"]
    in_maps = make_in_maps(x, w_attn, b_attn, w_proj)
    res = run_bass_kernel_spmd(nc, in_maps, list(range(N_CORES)))
    return combine_outputs(res.results, b_proj)


# revision 24
# speedup vs baseline: 1.4173x; 1.0369x over previous
"""Causal multi-head attention block (QKV proj -> attention -> out proj) on 8
Trainium2 NeuronCores.

Sharding: core i handles batch b = i//2 and head-group g = i%2 (6 of 12 heads).
Each core computes its heads' attention output and a partial output projection
(rows g*384:(g+1)*384 of w_proj); the host sums the two partials per batch and
adds b_proj.

On-core dataflow (per core):
  x^T tiles  [c,t]   via DMA-transpose (bf16)
  q^T, k^T   [n,t]   = w-stationary matmuls, pair-stacked 2 heads/tile
  v          [t,n]   natural layout, with a ones column appended per head
  S^T        [kt,qt] = k^T-stationary matmul, row-packed pairs (K=64 halves)
  P^T        = exp(S^T)  (no max subtraction: |scores| is O(10), safe in fp32)
  [out^T; l] [65,qt] = [v|1]-stationary matmul per head (l = softmax denom)
  out^T * (1/l)      -> ao^T [f,t], fed as lhsT to the projection matmul
Causal masking: only kt-tiles <= qt are computed; diagonal 128x128 blocks get a
0/1 triangular mask multiplied into P^T post-exp; fully-masked column ranges
are skipped via partial-free-dim matmuls into the same PSUM bank.

The emission order interleaves pair p+1's QKV matmul groups (and the final
projection) into pair p's attention stream: the attention phase is
ScalarE(exp)-bound, and PE executes in program order, so without interleaving
the PE sits idle between score/AV bursts, HAM re-throttles the clock to
1.2 GHz, and every matmul doubles in cost.
"""

import math
from contextlib import ExitStack

import numpy as np
import ml_dtypes

import concourse.bass as bass
import concourse.mybir as mybir
import concourse.tile as tile
from concourse import bacc, library_config
from concourse.bass_utils import run_bass_kernel_spmd

B, T_FULL, C = 4, 2048, 768
NH, HD = 12, 64
HL = NH // 2            # heads per core
NPAIR = HL // 2         # head pairs per core
NQK = HL * HD           # 384 features per core for each of q/k/v
N_CORES = 8
P = 128
SW = 512                # qt strip width
NC_T = C // P           # 6 contraction tiles
F32 = mybir.dt.float32
BF16 = mybir.dt.bfloat16
NPF = np.float32
NPBF = ml_dtypes.bfloat16

_CACHE: dict = {}


def build(T: int = T_FULL, interleave_on: bool = True):
    NT = T // P
    NSTRIP = T // SW
    nc = bacc.Bacc("TRN2", target_bir_lowering=False, debug=False,
                   num_devices=N_CORES)
    x_d = nc.dram_tensor("x", [T, C], BF16, kind="ExternalInput")
    w_d = nc.dram_tensor("wqkv", [C, 3 * NQK], BF16, kind="ExternalInput")
    bqk_d = nc.dram_tensor("bqk", [P, 2 * NPAIR], F32, kind="ExternalInput")
    bv_d = nc.dram_tensor("bv", [1, NQK], F32, kind="ExternalInput")
    wp_d = nc.dram_tensor("wp", [NQK, C], BF16, kind="ExternalInput")
    tri_d = nc.dram_tensor("tri", [P, P], F32, kind="ExternalInput")
    out_d = nc.dram_tensor("out", [T, C], F32, kind="ExternalOutput")

    EXP = mybir.ActivationFunctionType.Exp
    ADD = mybir.AluOpType.add
    MUL = mybir.AluOpType.mult

    with ExitStack() as ctx:
        tc = ctx.enter_context(tile.TileContext(nc))
        persist = ctx.enter_context(tc.tile_pool(name="persist", bufs=1))
        ppool = ctx.enter_context(tc.tile_pool(name="pt", bufs=6))
        smallp = ctx.enter_context(tc.tile_pool(name="small", bufs=4))
        outp = ctx.enter_context(tc.tile_pool(name="outsb", bufs=3))
        ps_s = ctx.enter_context(tc.tile_pool(name="ps_s", bufs=2, space="PSUM"))
        ps_q = ctx.enter_context(tc.tile_pool(name="ps_q", bufs=2, space="PSUM"))
        ps_av = ctx.enter_context(tc.tile_pool(name="ps_av", bufs=2, space="PSUM"))

        nc.gpsimd.load_library(library_config.attn)

        # ---- persistent inputs (spread across the two DMA-capable queues) ----
        w_sb = persist.tile([P, NC_T, 3 * NQK], BF16)
        nc.scalar.dma_start(w_sb[:], w_d.ap().rearrange("(a p) n -> p a n", p=P))
        tri_sb = persist.tile([P, P], F32)
        nc.sync.dma_start(tri_sb[:], tri_d.ap())
        bqk_sb = persist.tile([P, 2 * NPAIR], F32)
        nc.sync.dma_start(bqk_sb[:], bqk_d.ap())
        bv_bc = persist.tile([P, NQK], F32)
        bv_ap = bv_d.ap()
        bv_bcast = bass.AP(tensor=bv_ap.tensor, offset=bv_ap.offset,
                           ap=[[0, P], [1, NQK]])
        nc.sync.dma_start(bv_bc[:], bv_bcast)
        # x^T whole-column transposes (chunked slices raced with consumers)
        xT = persist.tile([P, NC_T, T], BF16)
        for cb in range(NC_T):
            eng = nc.sync if cb % 2 == 0 else nc.scalar
            eng.dma_start_transpose(xT[:, cb, :],
                                    x_d.ap()[:, cb * P:(cb + 1) * P])
        wp_sb = persist.tile([P, NQK // P, C], BF16)
        nc.sync.dma_start(wp_sb[:], wp_d.ap().rearrange("(a p) n -> p a n", p=P))

        # HAM warmup: keep PE busy with throwaway matmuls while x^T streams in
        warm_ps = ps_q.tile([P, P], F32, tag="q")
        for i in range(60):
            nc.tensor.matmul(warm_ps[:], tri_sb[:], tri_sb[:],
                             start=(i == 0), stop=(i == 59),
                             skip_group_check=True)

        # per-pair tensors (separate tiles so pair p+1 writes don't create
        # false deps against pair p reads)
        qT = [persist.tile([P, T], BF16, name=f"qT{i}", tag=f"qT{i}") for i in range(NPAIR)]
        kT = [persist.tile([P, T], BF16, name=f"kT{i}", tag=f"kT{i}") for i in range(NPAIR)]
        v_sb = [persist.tile([P, NT, 2, HD + 1], BF16, name=f"v{i}", tag=f"v{i}")
                for i in range(NPAIR)]
        aoT = [persist.tile([P, T], BF16, name=f"aoT{i}", tag=f"aoT{i}") for i in range(NPAIR)]
        for p in range(NPAIR):
            nc.vector.memset(v_sb[p][:, :, :, HD:HD + 1], 1.0)

        def qk_unit(p, is_k, s):
            dst = kT[p] if is_k else qT[p]
            # feature columns in w_sb: q pairs 0..2 then k pairs 0..2
            bidx = NPAIR + p if is_k else p
            fi = bidx * P
            ps_t = ps_q.tile([P, SW], F32, tag="q")
            for cb in range(NC_T):
                nc.tensor.matmul(
                    ps_t[:], w_sb[:, cb, fi:fi + P],
                    xT[:, cb, s * SW:(s + 1) * SW],
                    start=(cb == 0), stop=(cb == NC_T - 1))
            nc.scalar.activation(
                dst[:, s * SW:(s + 1) * SW], ps_t[:],
                mybir.ActivationFunctionType.Identity,
                bias=bqk_sb[:, bidx:bidx + 1], scale=1.0)

        def v_unit(p, tt):
            ps_t = ps_q.tile([P, SW], F32, tag="q")
            vcols = 2 * NQK + p * P
            for cb in range(NC_T):
                nc.tensor.matmul(
                    ps_t[:, 0:P], xT[:, cb, tt * P:(tt + 1) * P],
                    w_sb[:, cb, vcols:vcols + P],
                    start=(cb == 0), stop=(cb == NC_T - 1))
            nc.vector.tensor_tensor(
                out=v_sb[p][:, tt, :, 0:HD], in0=ps_t[:, 0:P],
                in1=bv_bc[:, p * P:(p + 1) * P], op=ADD)

        def qkv_units(p):
            us = []
            for is_k in (False, True):
                for s in range(NSTRIP):
                    us.append(lambda k=is_k, s=s: qk_unit(p, k, s))
            for tt in range(NT):
                us.append(lambda tt=tt: v_unit(p, tt))
            return us

        def proj_unit(tt):
            tts = slice(tt * P, (tt + 1) * P)
            ot = outp.tile([P, C], F32)
            for nch, n0, n1 in ((0, 0, SW), (1, SW, C)):
                pr = ps_q.tile([P, SW], F32, tag="q")
                for ft in range(NQK // P):
                    nc.tensor.matmul(pr[:, 0:n1 - n0], aoT[ft][:, tts],
                                     wp_sb[:, ft, n0:n1],
                                     start=(ft == 0), stop=(ft == NQK // P - 1))
                nc.vector.tensor_copy(ot[:, n0:n1], pr[:, 0:n1 - n0])
            nc.sync.dma_start(out_d.ap()[tts, :], ot[:])

        def attn_units(p):
            us = []
            state = {}

            def group_unit(s, g, n_kt):
                def run():
                    if g == 0:
                        state["avA"] = ps_av.tile([P, SW], F32, name="avA", tag="av")
                        state["avB"] = ps_av.tile([P, SW], F32, name="avB", tag="av")
                    avA, avB = state["avA"], state["avB"]
                    sA = ps_s.tile([P, 2, SW], F32, tag="s")
                    sB = ps_s.tile([P, 2, SW], F32, tag="s")
                    for sub in range(2):
                        kt = 2 * g + sub
                        j = kt - 4 * s
                        c0 = max(j, 0) * P
                        kts = slice(kt * P, (kt + 1) * P)
                        qts = slice(s * SW + c0, (s + 1) * SW)
                        nc.tensor.matmul(sA[:, sub, c0:SW], kT[p][0:64, kts],
                                         qT[p][0:64, qts], start=True,
                                         stop=True)
                        nc.tensor.matmul(sB[:, sub, c0:SW], kT[p][64:P, kts],
                                         qT[p][64:P, qts], start=True,
                                         stop=True)
                        if j >= 0:  # diagonal: -1e9 tri mask pre-exp
                            for st in (sA, sB):
                                nc.vector.tensor_tensor(
                                    out=st[:, sub, c0:c0 + P],
                                    in0=st[:, sub, c0:c0 + P],
                                    in1=tri_sb[:], op=ADD)
                    pA = ppool.tile([P, 2, SW], BF16, tag="pt")
                    pB = ppool.tile([P, 2, SW], BF16, tag="pt")
                    nc.scalar.activation(pA[:], sA[:], EXP)
                    nc.scalar.activation(pB[:], sB[:], EXP)
                    for sub in range(2):
                        kt = 2 * g + sub
                        c0 = max(kt - 4 * s, 0) * P
                        first, last = kt == 0, kt == n_kt - 1
                        nc.tensor.matmul(
                            avA[0:HD + 1, c0:SW], v_sb[p][:, kt, 0, :],
                            pA[:, sub, c0:SW], start=first, stop=last,
                            skip_group_check=True)
                        nc.tensor.matmul(
                            avB[0:HD + 1, c0:SW], v_sb[p][:, kt, 1, :],
                            pB[:, sub, c0:SW], start=first, stop=last,
                            skip_group_check=True)
                return run

            def norm_unit(s):
                def run():
                    avA, avB = state["avA"], state["avB"]
                    lA = smallp.tile([1, SW], F32, tag="lrow")
                    lB = smallp.tile([1, SW], F32, tag="lrow")
                    nc.vector.tensor_copy(lA[:], avA[HD:HD + 1, :])
                    nc.vector.tensor_copy(lB[:], avB[HD:HD + 1, :])
                    rlA = smallp.tile([1, SW], F32, tag="rl")
                    rlB = smallp.tile([1, SW], F32, tag="rl")
                    nc.vector.reciprocal_approx_fast(rlA[:], lA[:])
                    nc.vector.reciprocal_approx_fast(rlB[:], lB[:])
                    rbA = smallp.tile([HD, SW], F32, tag="rb")
                    rbB = smallp.tile([HD, SW], F32, tag="rb")
                    nc.gpsimd.partition_broadcast(rbA[:], rlA[:], channels=HD)
                    nc.gpsimd.partition_broadcast(rbB[:], rlB[:], channels=HD)
                    ss = slice(s * SW, (s + 1) * SW)
                    nc.vector.tensor_tensor(out=aoT[p][0:HD, ss],
                                            in0=avA[0:HD, :], in1=rbA[:],
                                            op=MUL)
                    nc.vector.tensor_tensor(out=aoT[p][HD:P, ss],
                                            in0=avB[0:HD, :], in1=rbB[:],
                                            op=MUL)
                return run

            for s in range(NSTRIP):
                n_kt = 4 * (s + 1)
                for g in range(n_kt // 2):
                    us.append(group_unit(s, g, n_kt))
                us.append(norm_unit(s))
            return us

        def interleave(a_units, b_units):
            """Emit a_units (attention, ACT-heavy) with b_units (PE-heavy)
            spread evenly between them. b_units are (min_idx, deadline, fn):
            fn may only be emitted after a_units[min_idx - 1], and MUST be
            emitted before a_units[deadline] (prerequisite of that unit)."""
            if not a_units:
                for _, _, u in b_units:
                    u()
                return
            ratio = len(b_units) / len(a_units)
            bi = 0
            for i, u in enumerate(a_units):
                while bi < len(b_units) and b_units[bi][1] <= i:
                    b_units[bi][2]()
                    bi += 1
                u()
                target = int(round((i + 1) * ratio))
                while bi < len(b_units) and bi < target \
                        and b_units[bi][0] <= i + 1:
                    b_units[bi][2]()
                    bi += 1
            while bi < len(b_units):
                b_units[bi][2]()
                bi += 1

        # attention(p) a-unit bookkeeping: strips contribute 2(s+1) group
        # units + 1 norm unit each; cum[s] = first a-index AFTER strip s
        BIG = 10 ** 9
        cum = {}
        acc = 0
        for s in range(NSTRIP):
            acc += 2 * (s + 1) + 1
            cum[s] = acc
        norm_done = cum

        # minimal prefix of qkv(0) so attention(0) strip 0 can start
        qk_unit(0, False, 0)
        qk_unit(0, True, 0)
        for tt in range(min(4, NT)):
            v_unit(0, tt)
        # rest of qkv(0), deadline-gated: strip s+1 prereqs must be emitted
        # before attention(0) strip s+1 begins (a-index cum[s])
        fill0 = []
        for s in range(1, NSTRIP):
            dl = cum[s - 1]
            fill0.append((0, dl, lambda s=s: qk_unit(0, False, s)))
            fill0.append((0, dl, lambda s=s: qk_unit(0, True, s)))
            for tt in range(4 * s, min(4 * s + 4, NT)):
                fill0.append((0, dl, lambda tt=tt: v_unit(0, tt)))
        for p in range(NPAIR):
            fill = list(fill0) if p == 0 else []
            if p + 1 < NPAIR:
                fill += [(0, BIG, u) for u in qkv_units(p + 1)]
            else:
                fill += [(norm_done[min(tt // 4, NSTRIP - 1)], BIG,
                          lambda tt=tt: proj_unit(tt)) for tt in range(NT)]
            if interleave_on:
                interleave(attn_units(p), fill)
            else:
                for _, dl, u in fill:
                    if dl < BIG:
                        u()
                for u in attn_units(p):
                    u()
                for _, dl, u in fill:
                    if dl >= BIG:
                        u()

    nc.compile()
    return nc


def make_in_maps(x, w_attn, b_attn, w_proj):
    """Shard the full inputs into per-core input maps (host side)."""
    scale = 1.0 / math.sqrt(HD)
    tri = np.where(np.arange(P)[:, None] <= np.arange(P)[None, :],
                   0.0, -1e9).astype(NPF)
    in_maps = []
    for core in range(N_CORES):
        b, g = divmod(core, 2)
        cs = slice(g * NQK, (g + 1) * NQK)
        wq = w_attn[:, 0 * C:1 * C][:, cs] * scale
        wk = w_attn[:, 1 * C:2 * C][:, cs]
        wv = w_attn[:, 2 * C:3 * C][:, cs]
        wqkv = np.concatenate([wq, wk, wv], axis=1).astype(NPBF)
        bq = b_attn[0 * C:1 * C][cs] * scale
        bk = b_attn[1 * C:2 * C][cs]
        bqk = np.ascontiguousarray(
            np.concatenate([bq, bk]).reshape(2 * NPAIR, P).T).astype(NPF)
        bv = b_attn[2 * C:3 * C][cs].astype(NPF).reshape(1, NQK)
        wp = w_proj[g * NQK:(g + 1) * NQK, :].astype(NPBF)
        in_maps.append({
            "x": np.ascontiguousarray(x[b]).astype(NPBF),
            "wqkv": wqkv, "bqk": bqk, "bv": bv, "wp": wp, "tri": tri,
        })
    return in_maps


def combine_outputs(results, b_proj):
    outs = [results[i]["out"] for i in range(N_CORES)]
    out = np.stack([outs[2 * b] + outs[2 * b + 1] for b in range(B)])
    return (out + b_proj[None, None, :].astype(NPF)).astype(NPF)


def kernel(x, w_attn, b_attn, w_proj, b_proj):
    x = np.asarray(x, dtype=NPF)
    w_attn = np.asarray(w_attn, dtype=NPF)
    b_attn = np.asarray(b_attn, dtype=NPF)
    w_proj = np.asarray(w_proj, dtype=NPF)
    b_proj = np.asarray(b_proj, dtype=NPF)
    if "nc" not in _CACHE:
        _CACHE["nc"] = build(T_FULL)
    nc = _CACHE["nc"]
    in_maps = make_in_maps(x, w_attn, b_attn, w_proj)
    res = run_bass_kernel_spmd(nc, in_maps, list(range(N_CORES)))
    return combine_outputs(res.results, b_proj)
